# revision 1
# baseline (speedup 1.0000x reference)
"""LocallyGroupedAttn (windowed attention, ws=7, 8 heads) on 8 trn2 cores.

Sharding: data-parallel over batch B=8, one image per NeuronCore.

Host-side prep (part of sharding/layout): x is cast to bf16 and laid out as
x^T slabs [slab, chalf, c(128), tau(896)] where tokens use pair-major order
    tau = 112*a + 49*wi + 7*k + j   (pair a, parity wi, col-in-window k,
                                     row j; 98 of 112 pair columns used)
so every window is a contiguous 49-column slice (matmul stationary operands
must have 1-D free access patterns) and qkv matmul streams stay unpermuted.
Weights are pre-transposed/cast on host as well; biases pre-broadcast.

Per-core pipeline per slab (16 slabs of 784 real tokens):
  load xT [128, 2, 896]
  q^T,k^T = wT-stationary matmuls (N=448)      -> ACT drain (+bias, bf16)
  v       = xT-window-stationary matmuls       -> DVE drain (+bias, bf16)
  scores^T[tk,tq] per (window,head), 4-head row-strip packing on PE
  exp on ACT (softmax scale folded into activation scale), bf16
  o_un[tq,d] + denom: PV matmul with ones-augmented v
  o = o_un * recip(denom) (DVE free-broadcast)  -> bf16
  o --DMA-transpose--> o^T ; proj matmuls ; +bias (DVE) ; scatter DMA out.
Window pairs sit at partition strips {0:49, 64:113} in every per-pair tile
(64 is a legal matmul output base partition; 49 is not).
"""

import json
import os

import numpy as np
import ml_dtypes

import concourse.bass as bass
import concourse.bass2jax as bass2jax
import concourse.tile as tile
from concourse import mybir
from concourse.bass_utils import compile_bir_kernel as _real_compile_bir_kernel
from concourse.bass_utils import run_bass_kernel_spmd


def _split_multi_waits(bir_bytes):
    """This container's walrus accepts at most ONE sync wait per instruction
    ("Too many sync wait commands"). Split extra waits onto standalone
    same-engine EventSemaphore wait carriers placed just before."""
    m = json.loads(bir_bytes)
    ctr = 0
    for f in m["functions"]:
        for blk in f["blocks"]:
            out = []
            for ins in blk.get("instructions", []):
                si = ins.get("sync_info")
                if si:
                    waits = si.get("on_wait") or []
                    if len(waits) > 1:
                        for wt in waits[:-1]:
                            ctr += 1
                            out.append({
                                "debug": ins.get("debug", 0),
                                "engine": ins["engine"],
                                "ins": [],
                                "outs": [],
                                "name": f"WSPLIT-{ctr}",
                                "opcode": "EventSemaphore",
                                "sync_info": {"on_update": [], "on_wait": [wt]},
                            })
                        si["on_wait"] = [waits[-1]]
                out.append(ins)
            blk["instructions"] = out
    return json.dumps(m).encode()


def _patched_compile_bir_kernel(bir_json, tmpdir, neff_name="file.neff"):
    if isinstance(bir_json, str):
        bir_json = bir_json.encode()
    return _real_compile_bir_kernel(_split_multi_waits(bir_json), tmpdir, neff_name)


bass2jax.compile_bir_kernel = _patched_compile_bir_kernel

F32 = mybir.dt.float32
BF16 = mybir.dt.bfloat16
AF = mybir.ActivationFunctionType
OP = mybir.AluOpType

B, H, W, C = 8, 112, 112, 256
WS, NH, HD = 7, 8, 32
N = H * W                     # 12544 tokens per image
SLAB_T = WS * W               # 784 real tokens per slab
NSLAB = H // WS               # 16
NWIN_ROW = W // WS            # 16 windows per slab
NPAIR = NWIN_ROW // 2         # 8 pairs per slab
WS2 = WS * WS                 # 49
PADT = 112 * NPAIR            # 896 padded tau columns per slab
SCALE = float(HD) ** -0.5


def build_bass() -> bass.Bass:
    nslab = int(os.environ.get("KBUILD_SLABS", NSLAB))
    ncouple = int(os.environ.get("KBUILD_COUPLES", NPAIR // 2))
    stage = int(os.environ.get("KBUILD_STAGE", "4"))
    repeat = int(os.environ.get("KBUILD_REPEAT", "1"))
    nc = bass.Bass()
    xT_in = nc.dram_tensor("xT", [NSLAB, 2, 128, PADT], BF16, kind="ExternalInput")
    wT_in = nc.dram_tensor("wT", [2, 128, 3 * C], BF16, kind="ExternalInput")
    wpT_in = nc.dram_tensor("wpT", [2, 128, C], BF16, kind="ExternalInput")
    cb_in = nc.dram_tensor("cb", [128, 4 + 2 * C], F32, kind="ExternalInput")
    out = nc.dram_tensor("out", [NSLAB, 113, NPAIR, C], F32, kind="ExternalOutput")

    with tile.TileContext(nc) as tc:
        with (
            tc.tile_pool(name="consts", bufs=1) as consts,
            tc.tile_pool(name="xt", bufs=3) as xt_pool,
            tc.tile_pool(name="qk", bufs=3) as qk_pool,
            tc.tile_pool(name="vt", bufs=4) as vt_pool,
            tc.tile_pool(name="es", bufs=6) as es_pool,
            tc.tile_pool(name="dn", bufs=6) as dn_pool,
            tc.tile_pool(name="ob", bufs=6) as o_pool,
            tc.tile_pool(name="ot", bufs=6) as ot_pool,
            tc.tile_pool(name="os", bufs=4) as out_pool,
            tc.tile_pool(name="ps", bufs=1, space="PSUM") as psum,
        ):
            # ---- constants -------------------------------------------------
            wT = consts.tile([128, 2, 3 * C], BF16)
            for ch in range(2):
                nc.gpsimd.dma_start(out=wT[:, ch, :], in_=wT_in[ch])
            wpT = consts.tile([128, 2, C], BF16)
            for ch in range(2):
                nc.gpsimd.dma_start(out=wpT[:, ch, :], in_=wpT_in[ch])
            cb = consts.tile([128, 4 + 2 * C], F32)
            nc.gpsimd.dma_start(out=cb, in_=cb_in[:, :])
            qkb = cb[:, 0:4]
            vb = cb[:, 4 : 4 + C]
            pb = cb[:, 4 + C : 4 + 2 * C]
            vts = []
            for i in range(4):
                vt_p = consts.tile([128, NH, HD + 1], BF16, name=f"vtp{i}")
                nc.gpsimd.memset(vt_p[:, :, HD : HD + 1], 1.0)
                vts.append(vt_p)

            # ---- main loop: software-pipelined across couples ----------
            # stage A(c): xT/qk loads + v-projs + qk^T matmuls + exp
            # stage B(c): PV + recip + o-norm + o^T transposes   (lag 1)
            # stage C(c): proj matmuls + drain into slab staging (lag 2)
            slab_state = {}

            def stage_a(gc):
                r, cp = divmod(gc, NPAIR // 2)
                if cp == 0:
                    xT = xt_pool.tile([128, 2, PADT], BF16, name=f"xT_{r}", tag="xt")
                    for ch in range(2):
                        nc.gpsimd.dma_start(out=xT[:, ch, :], in_=xT_in[r, ch])
                    qk_sb = qk_pool.tile([128, 4, PADT], BF16, name=f"qk_{r}", tag="qk")
                    for j in range(4):
                        for half in range(2):
                            ps = psum.tile([128, 448], F32, tag="mm", bufs=2)
                            for ch in range(2):
                                nc.tensor.matmul(
                                    ps,
                                    lhsT=wT[:, ch, 128 * j : 128 * (j + 1)],
                                    rhs=xT[:, ch, 448 * half : 448 * (half + 1)],
                                    start=(ch == 0),
                                    stop=(ch == 1),
                                )
                            nc.scalar.activation(
                                out=qk_sb[:, j, 448 * half : 448 * (half + 1)],
                                in_=ps,
                                func=AF.Identity,
                                bias=qkb[:, j : j + 1],
                                scale=1.0,
                            )
                    out_sb = out_pool.tile([128, NPAIR, C], F32, name=f"os_{r}", tag="os")
                    slab_state[r] = (xT, qk_sb, out_sb)
                xT, qk_sb, out_sb = slab_state[r]

                vt_c = []
                for idx, a in enumerate((2 * cp, 2 * cp + 1)):
                    ps_v = psum.tile([128, 448], F32, tag="mm", bufs=2)
                    for wi, b0 in ((0, 0), (1, 64)):
                        for ch in range(2):
                            nc.tensor.matmul(
                                ps_v[b0 : b0 + WS2, 0:C],
                                lhsT=xT[:, ch, 112 * a + 49 * wi :][:, 0:WS2],
                                rhs=wT[:, ch, 2 * C : 3 * C],
                                start=(ch == 0),
                                stop=(ch == 1),
                            )
                    vt = vts[(2 * cp + idx) % 4]
                    nc.vector.tensor_tensor(
                        vt[0:113, :, 0:HD],
                        ps_v[0:113, 0:C].rearrange("p (h d) -> p h d", h=NH),
                        vb[0:113, :].rearrange("p (h d) -> p h d", h=NH),
                        OP.add,
                    )
                    vt_c.append(vt)

                ps_s = psum.tile([128, 2048], F32, tag="s", bufs=1)
                for idx, a in enumerate((2 * cp, 2 * cp + 1)):
                    for wi, b0 in ((0, 0), (1, 64)):
                        tau0 = 112 * a + 49 * wi
                        for h in range(NH):
                            j = h // 4
                            p0 = 32 * (h % 4)
                            sc = 512 * (h % 4) + 98 * idx + WS2 * (h // 4)
                            nc.tensor.matmul(
                                ps_s[b0 : b0 + WS2, sc : sc + WS2],
                                lhsT=qk_sb[p0 : p0 + 32, 2 + j, tau0:][:, 0:WS2],
                                rhs=qk_sb[p0 : p0 + 32, j, tau0:][:, 0:WS2],
                                start=True,
                                stop=True,
                                tile_position=(p0, b0),
                            )
                es = es_pool.tile([128, 4 * 196], BF16)
                for sb in range(4):
                    nc.scalar.activation(
                        out=es[0:113, 196 * sb : 196 * (sb + 1)],
                        in_=ps_s[0:113, 512 * sb : 512 * sb + 196],
                        func=AF.Exp,
                        scale=SCALE,
                    )
                return {"es": es, "vt_c": vt_c, "r": r, "cp": cp}

            def stage_b(ctx):
                es, vt_c = ctx["es"], ctx["vt_c"]
                ps_oe = psum.tile([128, NH * (HD + 1)], F32, tag="oe", bufs=1)
                ps_oo = psum.tile([128, NH * (HD + 1)], F32, tag="oo", bufs=1)
                ps_par = [ps_oe, ps_oo]
                for idx in range(2):
                    r0 = 64 * idx
                    vt = vt_c[idx]
                    for par, e0 in ((0, 0), (1, 64)):
                        for h in range(NH):
                            ec = 196 * (h % 4) + 98 * idx + WS2 * (h // 4)
                            nc.tensor.matmul(
                                ps_par[par][r0 : r0 + WS2, 33 * h : 33 * (h + 1)],
                                lhsT=es[e0 : e0 + WS2, ec : ec + WS2],
                                rhs=vt[e0 : e0 + WS2, h, :],
                                start=True,
                                stop=True,
                            )
                oTs = []
                for par in range(2):
                    ps_o = ps_par[par]
                    den = dn_pool.tile([128, NH], F32)
                    nc.vector.reciprocal(
                        den[0:113, :],
                        ps_o[0:113, :].rearrange("p (h e) -> p h e", h=NH)[
                            :, :, HD : HD + 1
                        ],
                    )
                    o_sb = o_pool.tile([128, C], BF16)
                    nc.vector.tensor_tensor(
                        o_sb[0:113, :].rearrange("p (h d) -> p h d", h=NH),
                        ps_o[0:113, :].rearrange("p (h e) -> p h e", h=NH)[
                            :, :, 0:HD
                        ],
                        den[0:113, :, None].to_broadcast([113, NH, HD]),
                        OP.mult,
                    )
                    oT = ot_pool.tile([128, 2, 128], BF16)
                    for ch in range(2):
                        nc.sync.dma_start(
                            out=oT[:, ch, :],
                            in_=o_sb[:, 128 * ch : 128 * (ch + 1)],
                            transpose=True,
                        )
                    oTs.append(oT)
                ctx["oTs"] = oTs
                return ctx

            def stage_c(ctx):
                r, cp, oTs = ctx["r"], ctx["cp"], ctx["oTs"]
                out_sb = slab_state[r][2]
                for par in range(2):
                    oT = oTs[par]
                    ps_p = psum.tile([128, 448], F32, tag="mm", bufs=2)
                    for idx, b0 in ((0, 0), (1, 64)):
                        for ch in range(2):
                            nc.tensor.matmul(
                                ps_p[b0 : b0 + WS2, 0:C],
                                lhsT=oT[:, ch, 64 * idx : 64 * idx + WS2],
                                rhs=wpT[:, ch, :],
                                start=(ch == 0),
                                stop=(ch == 1),
                            )
                    nc.vector.tensor_tensor(
                        out_sb[0:113, 2 * cp + par, :],
                        ps_p[0:113, 0:C],
                        pb[0:113, :],
                        OP.add,
                    )
                if cp == NPAIR // 2 - 1:
                    nc.gpsimd.dma_start(out=out[r], in_=out_sb[0:113, :, :])
                    del slab_state[r]

            ncpl = NPAIR // 2
            total = nslab * ncpl
            ctxs = {}
            for gc in range(total + 2):
                if gc < total:
                    ctxs[gc] = stage_a(gc)
                if gc - 1 >= 0 and gc - 1 < total:
                    ctxs[gc - 1] = stage_b(ctxs[gc - 1])
                if gc - 2 >= 0:
                    stage_c(ctxs.pop(gc - 2))
    return nc


def _unscramble(o_perm):
    """[16, 113, 8, 256] staging -> [N, C]. Rows {0:49, 64:113} of
    parity-tile (2cp+par) are windows 4cp+par / 4cp+2+par in m=7k+j order."""
    o = np.empty((NSLAB, NWIN_ROW, WS, WS, C), dtype=o_perm.dtype)  # [s,w,k,j,c]
    for cp in range(4):
        for par in range(2):
            pt = o_perm[:, :, 2 * cp + par]
            o[:, 4 * cp + par] = pt[:, 0:WS2].reshape(NSLAB, WS, WS, C)
            o[:, 4 * cp + 2 + par] = pt[:, 64:113].reshape(NSLAB, WS, WS, C)
    # [s, w, k, j, c] -> t = 112j + 7w + k within slab
    o = o.transpose(0, 3, 1, 2, 4)  # [s, j, w, k, c]
    return o.reshape(N, C)


def slab_out(out, t0, w):
    """HBM rows of window w in (k, j, c) order matching out_sb row order."""
    return out[t0 : t0 + SLAB_T, :].rearrange(
        "(j w k) c -> w k j c", j=WS, w=NWIN_ROW, k=WS
    )[w]


def _prep_host(x, qkv_w, qkv_b, proj_w, proj_b):
    """Host-side layout prep. Returns (xT [B,16,2,128,896] bf16, shared)."""
    bf16 = ml_dtypes.bfloat16
    # x -> [B, slab, w, k, j, c] -> m = 7k + j pair-padded tau order
    xs = x.reshape(B, NSLAB, WS, NWIN_ROW, WS, C)
    xw = xs.transpose(0, 1, 3, 4, 2, 5).reshape(B, NSLAB, NWIN_ROW, WS2, C)
    xpad = np.zeros((B, NSLAB, NPAIR, 112, C), dtype=np.float32)
    xp = xw.reshape(B, NSLAB, NPAIR, 2, WS2, C)
    xpad[:, :, :, 0:WS2] = xp[:, :, :, 0]
    xpad[:, :, :, WS2 : 2 * WS2] = xp[:, :, :, 1]
    # [B, slab, tau, c] -> [B, slab, ch, c(128), tau]
    xt = xpad.reshape(B, NSLAB, PADT, C).transpose(0, 1, 3, 2)
    xt = xt.reshape(B, NSLAB, 2, 128, PADT).astype(bf16)
    xt = np.ascontiguousarray(xt)

    wT = np.ascontiguousarray(
        qkv_w.T.reshape(2, 128, 3 * C).astype(bf16)
    )
    wpT = np.ascontiguousarray(proj_w.T.reshape(2, 128, C).astype(bf16))
    cb = np.empty((128, 4 + 2 * C), dtype=np.float32)
    cb[:, 0:4] = qkv_b[0 : 2 * C].reshape(4, 128).T
    cb[:, 4 : 4 + C] = np.broadcast_to(qkv_b[2 * C :], (128, C))
    cb[:, 4 + C :] = np.broadcast_to(proj_b, (128, C))
    return xt, {"wT": wT, "wpT": wpT, "cb": cb}


_NC_CACHE = None


def _get_nc():
    global _NC_CACHE
    if _NC_CACHE is None:
        _NC_CACHE = build_bass()
    return _NC_CACHE


def kernel(x, qkv_w, qkv_b, proj_w, proj_b, H=None, W=None, **_ignored):
    x = np.ascontiguousarray(np.asarray(x, dtype=np.float32))
    assert x.shape == (B, N, C), x.shape
    xt, shared = _prep_host(
        x,
        np.asarray(qkv_w, dtype=np.float32),
        np.asarray(qkv_b, dtype=np.float32),
        np.asarray(proj_w, dtype=np.float32),
        np.asarray(proj_b, dtype=np.float32),
    )
    nc = _get_nc()
    in_maps = [{"xT": np.ascontiguousarray(xt[b]), **shared} for b in range(B)]
    res = run_bass_kernel_spmd(nc, in_maps, core_ids=list(range(B)))
    return np.stack(
        [_unscramble(r["out"]) for r in res.results], axis=0
    )


if __name__ == "__main__":
    rng = np.random.default_rng(0)
    inputs = {
        "x": rng.standard_normal((B, N, C), dtype=np.float32),
        "qkv_w": rng.standard_normal((3 * C, C), dtype=np.float32) / 16.0,
        "qkv_b": rng.standard_normal((3 * C,), dtype=np.float32) * 0.02,
        "proj_w": rng.standard_normal((C, C), dtype=np.float32) / 16.0,
        "proj_b": rng.standard_normal((C,), dtype=np.float32) * 0.02,
    }
    o = kernel(**inputs)
    print(o.shape, o.dtype, float(np.abs(o).mean()))

